# revision 29
# baseline (speedup 1.0000x reference)
"""Causal group-query attention on 8 Trainium2 NeuronCores.

Sharding: core c -> (batch b = c // 4, kv-group g = c % 4).
Each core owns batch element b, q-heads [4g, 4g+4) and kv-group g (n_rep = 4,
so those 4 q-heads attend to exactly kv-group g's k/v).  Every core computes
its partial o_proj output (contracting head-concat columns [512g, 512g+512)),
and the host sums the 4 partials per batch element (the "all-reduce after
o_proj" done host-side since we return full outputs anyway).

All matmul operands are fp16 (e5m10): same 1 cycle/row PE rate as fp32r at
N>=256 but no 4x penalty at small N, half the SBUF/DMA footprint, FWL weight
loads, and 1024-wide moving operands.  fp16 quantization (~0.03% RMS) keeps
the end-to-end rel err ~1e-3, far under the 2e-2 gate.  PSUM accumulation is
fp32 throughout.

Per-core kernel (T=2048, D=2048, HS=128):
  phase A (per 512-wide t-block): stream x^T fp16, q projections in two
    head-pair passes (each into one 2-bank PSUM tile), RoPE per head on
    ACT/DVE with the rotate-half permutation as a 128x128 matmul; then k/v
    projections (shared 2-bank tile), k-RoPE, v bias + transpose to [t, hs]
    fp16 tiles.  The previous block's o_proj is emitted after the k/v pass so
    the PE chews on it while ACT/DVE finish the RoPE chains.
  phase B: heads processed in pairs with merged matmuls: for each causally
    valid 128-wide k-tile i, ONE S matmul per pair (moving qf [128, 2, N'],
    output S^T pair tile [128, 2, 512] = 2 PSUM banks), ONE exp ACTIVATE over
    both heads, triangular-mask multiplies on diagonal subtiles (DVE), then
    ONE PV matmul (moving pt pair) and two 1-row denominator matmuls per
    tile, all PSUM-accumulated.  Software-pipelined by one k-tile so the PE
    never waits on the S->exp chain.
  normalize: denominator rows evacuate via ACT, broadcast across partitions
    with a K=1 ones matmul, reciprocal_approx_fast on DVE (~5x faster than
    exact reciprocal; 18-bit accuracy), multiply into the evacuated O^T pair.
  phase C: o_proj out[tq, d] = sum_h O^T_h-stationary @ Wo^T_h moving, fp16
    out tiles DMA'd to DRAM; host upcasts to f32 and sums the 4 partials.

PSUM plan (8 banks): tag "pb" = 3 bufs of [128, 2, 512] f32 (6 banks) rotating
through qt-pair x2 / ktvt / S-pair x2 / O-pair; tag "b1" = 2 bufs of
[128, 512] f32 (2 banks) for rot/vtransp/oproj/denominator/broadcast.
"""

import math

import numpy as np

B, T, D = 2, 2048, 2048
N_HEAD, N_GROUP = 16, 4
HS = D // N_HEAD  # 128
N_REP = N_HEAD // N_GROUP  # 4
NH_C = N_HEAD // N_GROUP  # heads per core = 4
INV_SQRT_HS = 1.0 / math.sqrt(HS)

_NC_CACHE: dict = {}


def build_nc(t=T, gp_outdma=True, dve_den=True):
    """Build and compile the per-core Bass program. Returns the compiled nc."""
    import concourse.bass as bass  # noqa: F401
    import concourse.mybir as mybir
    import concourse.tile as tile
    from concourse import bacc

    f32 = mybir.dt.float32
    f16 = mybir.dt.float16
    ident_f = mybir.ActivationFunctionType.Identity
    exp_f = mybir.ActivationFunctionType.Exp

    nd = D // 128  # d-tiles (contraction) = 16
    tb_n = t // 512  # 512-wide t blocks
    nk = t // 128  # 128-wide k tiles

    nc = bacc.Bacc("TRN2", target_bir_lowering=False, debug=False)

    xd = nc.dram_tensor("x_t", [D, t], f16, kind="ExternalInput")
    wqd = nc.dram_tensor("wq_t", [D, NH_C * HS], f16, kind="ExternalInput")
    wkd = nc.dram_tensor("wk_t", [D, HS], f16, kind="ExternalInput")
    wvd = nc.dram_tensor("wv_t", [D, HS], f16, kind="ExternalInput")
    wod = nc.dram_tensor("wo_t", [NH_C * HS, D], f16, kind="ExternalInput")
    cosd = nc.dram_tensor("cos_t", [HS, t], f16, kind="ExternalInput")
    sind = nc.dram_tensor("sin_t", [HS, t], f16, kind="ExternalInput")
    bqd = nc.dram_tensor("b_q", [HS, NH_C], f32, kind="ExternalInput")
    bkd = nc.dram_tensor("b_k", [HS, 1], f32, kind="ExternalInput")
    bvd = nc.dram_tensor("b_v", [HS, 1], f32, kind="ExternalInput")
    rtd = nc.dram_tensor("r_t", [HS, HS], f16, kind="ExternalInput")
    maskd = nc.dram_tensor("mask_ut", [128, 128], f16, kind="ExternalInput")
    identd = nc.dram_tensor("ident", [128, 128], f16, kind="ExternalInput")
    outd = nc.dram_tensor("out", [t, D], f16, kind="ExternalOutput")

    with tile.TileContext(nc) as tc:
        with (
            tc.tile_pool(name="consts", bufs=1) as consts,
            tc.tile_pool(name="wpool", bufs=1) as wpool,
            tc.tile_pool(name="resid", bufs=1) as resid,
            tc.tile_pool(name="xin", bufs=16) as xin,
            tc.tile_pool(name="work", bufs=3) as work,
            tc.tile_pool(name="ptp", bufs=8) as ptp,
            tc.tile_pool(name="qfp", bufs=2) as qfp,
            tc.tile_pool(name="otp", bufs=5) as otp,
            tc.tile_pool(name="outp", bufs=6) as outp,
            tc.tile_pool(name="psum", bufs=3, space="PSUM") as psum,
        ):
            def pb(name):
                # qt pairs + S pair tiles (2-deep pipeline)
                return psum.tile([128, 2, 512], f32, tag="pb", bufs=2, name=name)

            def po(name):
                # ktvt + O pair tiles: own tag so the qt/S rotation never
                # lands on a tile still waiting on the normalize chain
                return psum.tile([128, 2, 512], f32, tag="po", bufs=1, name=name)

            def b1(name):
                return psum.tile([128, 512], f32, tag="b1", bufs=2, name=name)

            # ---- constants / weights (loaded once) ----
            cos_sb = consts.tile([128, t], f16, name="cos_sb")
            sin_sb = consts.tile([128, t], f16, name="sin_sb")
            rt_sb = consts.tile([128, 128], f16, name="rt_sb")
            mask_sb = consts.tile([128, 128], f16, name="mask_sb")
            id_sb = consts.tile([128, 128], f16, name="id_sb")
            ones_f = consts.tile([128, 128], f32, name="ones_f")
            ones_sb = consts.tile([128, 128], f16, name="ones_sb")
            bq_sb = consts.tile([128, NH_C], f32, name="bq_sb")
            bk_sb = consts.tile([128, 1], f32, name="bk_sb")
            bv_sb = consts.tile([128, 1], f32, name="bv_sb")
            wq_sb = wpool.tile([128, nd, NH_C * HS], f16, name="wq_sb")
            wk_sb = wpool.tile([128, nd, HS], f16, name="wk_sb")
            wv_sb = wpool.tile([128, nd, HS], f16, name="wv_sb")
            wo_sb = wpool.tile([128, NH_C, D], f16, name="wo_sb")
            wq_re = wqd[:, :].rearrange("(n p) m -> p n m", p=128)
            wk_re = wkd[:, :].rearrange("(n p) m -> p n m", p=128)
            wv_re = wvd[:, :].rearrange("(n p) m -> p n m", p=128)

            # resident K^T [hs, t] and V [t(128-tiles), hs]
            kt_sb = resid.tile([128, t], f16, name="kt_sb")
            v_sb = resid.tile([128, nk, HS], f16, name="v_sb")

            x_re = xd[:, :].rearrange("(n p) t -> p n t", p=128)

            def emit_op_tile(tb, ot_sb, s, db):
                # one o_proj output tile [128 tq, 512 d] for q-block tb
                ts0 = tb * 512
                op_ps = b1("op_ps")
                for h in range(NH_C):
                    nc.tensor.matmul(
                        op_ps,
                        lhsT=ot_sb[h // 2][:, h % 2, 128 * s : 128 * (s + 1)],
                        rhs=wo_sb[:, h, 512 * db : 512 * (db + 1)],
                        start=h == 0, stop=h == NH_C - 1,
                    )
                ob = outp.tile([128, 512], f16, name="ob")
                if (s + db) % 2 == 0:
                    nc.vector.tensor_copy(out=ob, in_=op_ps)
                else:
                    nc.scalar.copy(out=ob, in_=op_ps)
                # alternate DMA queues so writes drain on two engine queues
                dma_eng = (
                    nc.gpsimd if (gp_outdma and (s + db) % 2 == 0) else nc.sync
                )
                dma_eng.dma_start(
                    out=outd[
                        ts0 + 128 * s : ts0 + 128 * (s + 1),
                        512 * db : 512 * (db + 1),
                    ],
                    in_=ob,
                )

            def emit_oproj_final(tb, ot_sb):
                # last block's o_proj: every PSUM bank is free now, so use
                # 2-tile pair PSUM tiles (3-deep rotation across pb/po), one
                # evacuation and one contiguous [128, 1024] DMA per pair --
                # the per-tile b1 evac round-trips otherwise stall the PE
                # ~1us/tile here, with nothing left to overlap them.
                ts0 = tb * 512
                k = 0
                for s in range(4):
                    for db in range(0, D // 512, 2):
                        op2 = pb("op2") if k % 3 < 2 else po("op2")
                        for half in range(2):
                            for h in range(NH_C):
                                nc.tensor.matmul(
                                    op2[:, half, :],
                                    lhsT=ot_sb[h // 2][
                                        :, h % 2, 128 * s : 128 * (s + 1)
                                    ],
                                    rhs=wo_sb[
                                        :, h,
                                        512 * (db + half) : 512 * (db + half + 1),
                                    ],
                                    start=h == 0, stop=h == NH_C - 1,
                                    skip_group_check=True,
                                )
                        ob2 = outp.tile([128, 2, 512], f16, name="ob2")
                        if k % 2 == 0:
                            nc.vector.tensor_copy(out=ob2, in_=op2)
                        else:
                            nc.scalar.copy(out=ob2, in_=op2)
                        dma_eng = nc.gpsimd if (gp_outdma and k % 2 == 0) else nc.sync
                        dma_eng.dma_start(
                            out=outd[
                                ts0 + 128 * s : ts0 + 128 * (s + 1),
                                512 * db : 512 * (db + 2),
                            ],
                            in_=ob2,
                        )
                        k += 1

            def rope(dst, raw, rot_ps, ts0):
                # dst = raw * cos + (R @ raw) * sin   (rot_ps already in PSUM)
                nc.vector.tensor_mul(dst, raw, cos_sb[:, ts0 : ts0 + 512])
                rtmp = work.tile([128, 512], f16, name="rtmp", bufs=2)
                nc.vector.tensor_mul(rtmp, rot_ps, sin_sb[:, ts0 : ts0 + 512])
                nc.vector.tensor_add(dst, dst, rtmp)

            pending_oproj = None
            for tb in range(tb_n):
                ts0 = tb * 512
                # ============ phase A: projections + RoPE for this t-block
                # q heads in two pair-passes, then k/v, so at most two 2-bank
                # PSUM tiles are alive at once.
                xts = []
                for chunk in range(nd // 2):
                    c2 = 2 * chunk
                    xt = xin.tile([128, 2, 512], f16, name="xt")
                    nc.sync.dma_start(
                        out=xt, in_=x_re[:, c2 : c2 + 2, ts0 : ts0 + 512]
                    )
                    xts.append(xt)
                    if chunk == 0:
                        # per-block cos/sin slices: only this block's 512
                        # columns are needed, keeping 768KB of the full
                        # tables out of the startup HBM window
                        nc.gpsimd.dma_start(
                            out=cos_sb[:, ts0 : ts0 + 512],
                            in_=cosd[:, ts0 : ts0 + 512],
                        )
                        nc.gpsimd.dma_start(
                            out=sin_sb[:, ts0 : ts0 + 512],
                            in_=sind[:, ts0 : ts0 + 512],
                        )
                    if tb == 0:
                        nc.sync.dma_start(
                            out=wq_sb[:, c2 : c2 + 2, :], in_=wq_re[:, c2 : c2 + 2, :]
                        )
                        nc.sync.dma_start(
                            out=wk_sb[:, c2 : c2 + 2, :], in_=wk_re[:, c2 : c2 + 2, :]
                        )
                        nc.sync.dma_start(
                            out=wv_sb[:, c2 : c2 + 2, :], in_=wv_re[:, c2 : c2 + 2, :]
                        )
                        if chunk == 0:
                            nc.sync.dma_start(out=bq_sb, in_=bqd[:, :])
                            nc.sync.dma_start(out=bk_sb, in_=bkd[:, :])
                            nc.sync.dma_start(out=bv_sb, in_=bvd[:, :])
                            nc.vector.memset(ones_f, 1.0)
                            nc.scalar.copy(out=ones_sb, in_=ones_f)
                            # one-time loads on the gpsimd queue so the sync
                            # queue stays dedicated to the xt/w stream
                            nc.gpsimd.dma_start(out=rt_sb, in_=rtd[:, :])
                            nc.gpsimd.dma_start(out=mask_sb, in_=maskd[:, :])
                            nc.gpsimd.dma_start(out=id_sb, in_=identd[:, :])
                if tb == min(1, tb_n - 1):
                    # wo is first consumed by block 1's o_proj emission; defer
                    # its 2MB load out of the startup HBM contention window
                    wo_re = wod[:, :].rearrange("(h p) m -> p h m", p=128)
                    for h in range(NH_C):
                        nc.gpsimd.dma_start(
                            out=wo_sb[:, h : h + 1, :],
                            in_=wo_re[:, h : h + 1, :],
                        )

                # projection matmuls: block 0 interleaves all six per chunk
                # (the PE is paced by the x/w DMA stream there — keep it fed
                # the moment each chunk lands); later blocks run pass-split
                # so the PE streams without waiting on any ACT/DVE evac.
                qt2 = {0: pb("qt2_0"), 1: pb("qt2_1")}
                ktvt = po("ktvt")
                if tb == 0:
                    for chunk in range(nd // 2):
                        for j in range(2):
                            dt = 2 * chunk + j
                            first, last = dt == 0, dt == nd - 1
                            for h in range(NH_C):
                                nc.tensor.matmul(
                                    qt2[h // 2][:, h % 2, :],
                                    lhsT=wq_sb[:, dt, h * HS : (h + 1) * HS],
                                    rhs=xts[chunk][:, j, :],
                                    start=first, stop=last,
                                    skip_group_check=True,
                                )
                            nc.tensor.matmul(
                                ktvt[:, 0, :], lhsT=wk_sb[:, dt, :],
                                rhs=xts[chunk][:, j, :],
                                start=first, stop=last, skip_group_check=True,
                            )
                            nc.tensor.matmul(
                                ktvt[:, 1, :], lhsT=wv_sb[:, dt, :],
                                rhs=xts[chunk][:, j, :],
                                start=first, stop=last, skip_group_check=True,
                            )
                else:
                    for p in range(2):
                        for chunk in range(nd // 2):
                            for j in range(2):
                                dt = 2 * chunk + j
                                first, last = dt == 0, dt == nd - 1
                                for hh in range(2):
                                    h = 2 * p + hh
                                    nc.tensor.matmul(
                                        qt2[p][:, hh, :],
                                        lhsT=wq_sb[:, dt, h * HS : (h + 1) * HS],
                                        rhs=xts[chunk][:, j, :],
                                        start=first, stop=last,
                                        skip_group_check=True,
                                    )
                    for chunk in range(nd // 2):
                        for j in range(2):
                            dt = 2 * chunk + j
                            first, last = dt == 0, dt == nd - 1
                            nc.tensor.matmul(
                                ktvt[:, 0, :], lhsT=wk_sb[:, dt, :],
                                rhs=xts[chunk][:, j, :],
                                start=first, stop=last, skip_group_check=True,
                            )
                            nc.tensor.matmul(
                                ktvt[:, 1, :], lhsT=wv_sb[:, dt, :],
                                rhs=xts[chunk][:, j, :],
                                start=first, stop=last, skip_group_check=True,
                            )

                # previous block's o_proj: first half emitted here (fills the
                # PE while ACT evacuates qt2 and the rope chains run), second
                # half interleaved into phase B (fills the ~300ns/tile exp
                # latency bubbles there).
                op_queue = (
                    [(s, db) for s in range(4) for db in range(D // 512)]
                    if pending_oproj is not None else []
                )
                for s, db in op_queue[:8]:
                    emit_op_tile(tb - 1, pending_oproj, s, db)
                op_queue = op_queue[8:]
                prev_ot = pending_oproj

                qf = qfp.tile([128, NH_C, 512], f16, name="qf")
                for p in range(2):
                    for hh in range(2):
                        h = 2 * p + hh
                        qraw = work.tile([128, 512], f16, name="qraw")
                        nc.scalar.activation(
                            out=qraw, in_=qt2[p][:, hh, :], func=ident_f,
                            bias=bq_sb[:, h : h + 1], scale=1.0,
                        )
                        rot_ps = b1("rot_ps")
                        nc.tensor.matmul(
                            rot_ps, lhsT=rt_sb, rhs=qraw, start=True, stop=True
                        )
                        rope(qf[:, h, :], qraw, rot_ps, ts0)

                kraw = work.tile([128, 512], f16, name="qraw")
                nc.scalar.activation(
                    out=kraw, in_=ktvt[:, 0, :], func=ident_f,
                    bias=bk_sb[:, 0:1], scale=1.0,
                )
                rot_ps = b1("rot_ps")
                nc.tensor.matmul(rot_ps, lhsT=rt_sb, rhs=kraw, start=True, stop=True)
                rope(kt_sb[:, ts0 : ts0 + 512], kraw, rot_ps, ts0)

                # v: bias (fp16 cast), then transpose to [t, hs] tiles
                vraw = work.tile([128, 512], f16, name="qraw")
                nc.scalar.activation(
                    out=vraw, in_=ktvt[:, 1, :], func=ident_f,
                    bias=bv_sb[:, 0:1], scale=1.0,
                )
                for s in range(4):
                    vt_tp = b1("vt_tp")
                    vt16 = vt_tp[:, 0:64].bitcast(f16)
                    nc.tensor.transpose(
                        vt16, vraw[:, 128 * s : 128 * (s + 1)], id_sb
                    )
                    # fp16->fp16 copy on DVE (2x 16-bit rate) keeps ACT free
                    # for the exp stream
                    nc.vector.tensor_copy(out=v_sb[:, 4 * tb + s, :], in_=vt16)

                # ============ phase B: attention for q-block jq == tb
                # Pair-merged: one S matmul / exp / PV matmul per head-pair
                # and k-tile.  Software-pipelined by one k-tile.
                ot_sb = {}
                imax = 4 * tb + 3
                for p in range(2):
                    ot2 = po(f"ot2_{p}")
                    den = None if dve_den else b1(f"den_{p}")
                    if dve_den:
                        # fp16 accumulator: 2x DVE rate, and the denominator
                        # matmul reads it directly (no cast on the tail chain)
                        acc = work.tile(
                            [128, 2, 512], f16, name="acc", bufs=2
                        )

                    def emit_pv_den(i, pt, c0, ot2=ot2, den=den):
                        first, last = i == 0, i == imax
                        for hh in range(2):
                            nc.tensor.matmul(
                                ot2[:, hh, c0:], lhsT=v_sb[:, i, :],
                                rhs=pt[:, hh, c0:],
                                start=first, stop=last, skip_group_check=True,
                            )
                        if not dve_den:
                            for hh in range(2):
                                nc.tensor.matmul(
                                    den[32 * hh : 32 * hh + 1, c0:],
                                    lhsT=ones_sb[:, 0:1],
                                    rhs=pt[:, hh, c0:],
                                    start=first, stop=last,
                                    skip_group_check=True,
                                )

                    prev = None
                    for i in range(imax + 1):
                        c0 = 128 * max(0, i - 4 * tb)
                        st2 = pb("st2")
                        for hh in range(2):
                            nc.tensor.matmul(
                                st2[:, hh, c0:],
                                lhsT=kt_sb[:, 128 * i : 128 * (i + 1)],
                                rhs=qf[:, 2 * p + hh, c0:],
                                start=True, stop=True, skip_group_check=True,
                            )
                        pt = ptp.tile([128, 2, 512], f16, name="pt")
                        nc.scalar.activation(
                            out=pt[:, :, c0:], in_=st2[:, :, c0:], func=exp_f,
                            scale=INV_SQRT_HS,
                        )
                        if i >= 4 * tb:
                            for hh in range(2):
                                nc.vector.tensor_mul(
                                    pt[:, hh, c0 : c0 + 128],
                                    pt[:, hh, c0 : c0 + 128],
                                    mask_sb,
                                )
                        if dve_den:
                            # running P column-sum on DVE: frees the PE from
                            # the 1-row denominator matmuls per k-tile
                            if i == 0:
                                nc.vector.tensor_copy(out=acc, in_=pt)
                            else:
                                nc.vector.tensor_add(
                                    acc[:, :, c0:], acc[:, :, c0:], pt[:, :, c0:]
                                )
                        if prev is not None:
                            emit_pv_den(*prev)
                            if op_queue:
                                emit_op_tile(tb - 1, prev_ot, *op_queue.pop(0))
                        prev = (i, pt, c0)
                    emit_pv_den(*prev)
                    if dve_den:
                        # one pair of 1-row reductions over the accumulated
                        # P-sums instead of one per k-tile
                        den = b1(f"den_{p}")
                        for hh in range(2):
                            nc.tensor.matmul(
                                den[32 * hh : 32 * hh + 1, :],
                                lhsT=ones_sb[:, 0:1],
                                rhs=acc[:, hh, :],
                                start=True, stop=True,
                                skip_group_check=True,
                            )

                    # normalize: broadcast the raw denominator with a K=1
                    # ones matmul, fast-approx reciprocal on DVE, multiply.
                    # Mid-kernel pairs evacuate O^T via ACT first so the PSUM
                    # tile frees fast; the final pair multiplies straight from
                    # PSUM (nothing waits on that tile at kernel end) to keep
                    # the tail chain short.
                    last_pair = tb == tb_n - 1 and p == 1
                    osb = otp.tile([128, 2, 512], f16, name="osb")
                    if not last_pair:
                        nc.scalar.copy(out=osb, in_=ot2)
                    denrow = work.tile([33, 512], f16, name="denrow", bufs=2)
                    nc.scalar.copy(out=denrow[0:1, :], in_=den[0:1, :])
                    nc.scalar.copy(out=denrow[32:33, :], in_=den[32:33, :])
                    for hh in range(2):
                        bc_ps = b1("bc_ps")
                        nc.tensor.matmul(
                            bc_ps,
                            lhsT=ones_sb[32 * hh : 32 * hh + 1, 0:128],
                            rhs=denrow[32 * hh : 32 * hh + 1, :],
                            start=True, stop=True,
                        )
                        bcr = work.tile([128, 512], f32, name="bcr", bufs=2)
                        nc.vector.reciprocal_approx_fast(out=bcr, in_=bc_ps)
                        if last_pair:
                            nc.vector.tensor_mul(
                                osb[:, hh, :], ot2[:, hh, :], bcr
                            )
                        else:
                            nc.vector.tensor_mul(
                                osb[:, hh, :], osb[:, hh, :], bcr
                            )
                    ot_sb[p] = osb

                # flush any o_proj tiles not consumed by the interleave
                # (only happens for the smallest blocks)
                for s, db in op_queue:
                    emit_op_tile(tb - 1, prev_ot, s, db)
                pending_oproj = ot_sb

            emit_oproj_final(tb_n - 1, pending_oproj)

    nc.compile()
    return nc


def shard_inputs(x, cos, sin, Wq, bq, Wkv, bkv, Wo, t=T):
    """Build the 8 per-core input maps (core c -> batch c//4, group c%4)."""
    f16 = np.float16
    f32 = np.float32
    hs = HS
    rot = np.zeros((hs, hs), f32)
    for i in range(hs // 2):
        rot[i, i + hs // 2] = -1.0
        rot[i + hs // 2, i] = 1.0
    r_t = np.ascontiguousarray(rot.T.astype(f16))
    mask_ut = np.triu(np.ones((128, 128), f16))
    ident = np.eye(128, dtype=f16)
    cos_t = np.ascontiguousarray(np.asarray(cos, f32).T.astype(f16))
    sin_t = np.ascontiguousarray(np.asarray(sin, f32).T.astype(f16))

    xts = [
        np.ascontiguousarray(np.asarray(x[b], f32).T.astype(f16))
        for b in range(x.shape[0])
    ]
    per_g = []
    for g in range(4):
        per_g.append(
            dict(
                wq_t=np.ascontiguousarray(
                    Wq[512 * g : 512 * g + 512].T.astype(f16)
                ),
                b_q=np.ascontiguousarray(
                    bq[512 * g : 512 * g + 512].reshape(4, 128).T.astype(f32)
                ),
                wk_t=np.ascontiguousarray(
                    Wkv[128 * g : 128 * g + 128].T.astype(f16)
                ),
                b_k=np.ascontiguousarray(
                    bkv[128 * g : 128 * g + 128].reshape(128, 1).astype(f32)
                ),
                wv_t=np.ascontiguousarray(
                    Wkv[512 + 128 * g : 512 + 128 * g + 128].T.astype(f16)
                ),
                b_v=np.ascontiguousarray(
                    bkv[512 + 128 * g : 512 + 128 * g + 128]
                    .reshape(128, 1)
                    .astype(f32)
                ),
                wo_t=np.ascontiguousarray(
                    Wo[:, 512 * g : 512 * g + 512].T.astype(f16)
                ),
            )
        )

    in_maps = []
    for c in range(4 * x.shape[0]):
        b, g = c // 4, c % 4
        m = dict(per_g[g])
        m.update(
            x_t=xts[b], cos_t=cos_t, sin_t=sin_t,
            r_t=r_t, mask_ut=mask_ut, ident=ident,
        )
        in_maps.append(m)
    return in_maps


def run_on_hw(in_maps, t=T, trace=False, **flags):
    from concourse.bass_utils import run_bass_kernel_spmd

    key = (t, tuple(sorted(flags.items())))
    if key not in _NC_CACHE:
        _NC_CACHE[key] = build_nc(t, **flags)
    nc = _NC_CACHE[key]
    res = run_bass_kernel_spmd(
        nc, in_maps, core_ids=list(range(len(in_maps))), trace=trace
    )
    return res


def kernel(x, cos, sin, Wq, bq, Wkv, bkv, Wo):
    x = np.asarray(x)
    in_maps = shard_inputs(
        x, np.asarray(cos), np.asarray(sin), np.asarray(Wq), np.asarray(bq),
        np.asarray(Wkv), np.asarray(bkv), np.asarray(Wo),
    )
    res = run_on_hw(in_maps, t=T, trace=False)
    out = np.zeros((B, T, D), np.float32)
    for c, rmap in enumerate(res.results):
        out[c // 4] += rmap["out"].astype(np.float32)
    return out
